# revision 3
# baseline (speedup 1.0000x reference)
"""Trainium2 Bass kernel for nn_AttentionMask_13048110645633.

Math: for key (4,32,64,64) and query (4,512), with s = key.reshape(B,J) and
q = query, the reference computes per element

    ctx[b,j] = sum_k q[b,k]*exp(s[b,j]*q[b,k]) / sum_k exp(s[b,j]*q[b,k])
    out[b,j] = s[b,j] * sigmoid(ctx[b,j])

i.e. out = s * g_b(s) where g_b is a smooth scalar function determined by
q[b].  Sharding: data-parallel over B (4 batches x 2 half-slabs = 8 cores),
each core owns one (128,512) tile.

Device program (per core):
  1. PE-broadcast q to all 128 partitions (exact via an fp16 hi/lo pair
     summed by a single C=2 matmul into fp32 PSUM).
  2. Gate fit at 64 Chebyshev nodes s_n using the delta=1/2 log-sum-exp
     identity  g(s) ~= S0(s+1/2) / (S0(s+1/2) + S0(s-1/2))  where
     S0(s) = sum_k e^{s q_k}  (error ~2.5e-3, sigmoid folds away
     algebraically).  ONE ACT exp with per-partition scales
     [s_n+1/2 ; s_n-1/2] (64 nodes stacked twice across 128 partitions)
     + fused accumulate gives both S0 vectors; add/reciprocal/mult on DVE.
  3. PE-contract the 64 node gates with a precomputed pinv(Vandermonde)
     fit matrix -> monomial coefficients c0..c5 in t = tanh(s/2),
     replicated on all 128 partitions.
  4. Element path in fp16: t = ACT tanh, u = t^2, then Horner in u with
     odd/even interleave  p = L0 + u*(L1 + u*L2),  L_i = c_{2i} + c_{2i+1} t
     (tensor_scalar runs in 4x fp16 mode with per-partition coefficient
     ptrs); out = p * s.  Op order interleaves independent terms between
     dependent links and column-splits the tail so no DVE op pays the
     write-commit stall.
Output fp16, host casts to fp32 (well within the 2e-2 gate; measured
rel err ~3.6e-3).
"""

import numpy as np

B, J, K = 4, 131072, 512
P, F = 128, 512
NCORES = 8
D = 5
NN = 64
WARP_A = 0.5
SRANGE = 5.5
DELTA = 0.5

_CONSTS = None
_NC = None


def _host_constants():
    global _CONSTS
    if _CONSTS is not None:
        return _CONSTS
    tmax = float(np.tanh(WARP_A * SRANGE))
    th = (np.arange(NN) + 0.5) * np.pi / NN
    un = np.cos(th)
    sn = np.arctanh(un * tmax) / WARP_A          # node s-values
    tn = un * tmax
    V = np.vander(tn, D + 1, increasing=True)    # (NN, D+1)
    G = np.linalg.pinv(V)                        # (D+1, NN)
    cst = np.zeros((P, 8), np.float32)
    cst[:NN, 0] = (sn + DELTA).astype(np.float32)
    cst[NN:, 0] = (sn - DELTA).astype(np.float32)
    cst[:NN, 1:D + 2] = G.T.astype(np.float32)   # (NN, D+1)
    _CONSTS = cst
    return cst


def _build_nc():
    import concourse.bacc as bacc
    import concourse.mybir as mybir
    from concourse import tile

    fp32 = mybir.dt.float32
    fp16 = mybir.dt.float16
    AF = mybir.ActivationFunctionType
    OP = mybir.AluOpType

    nc = bacc.Bacc("TRN2", target_bir_lowering=False, debug=False,
                   num_devices=NCORES)
    s_d = nc.dram_tensor("s16", (P, F), fp16, kind="ExternalInput")
    qp_d = nc.dram_tensor("qpair", (2, K), fp16, kind="ExternalInput")
    cst_d = nc.dram_tensor("cst", (P, 8), fp32, kind="ExternalInput")
    y_d = nc.dram_tensor("y", (P, F), fp16, kind="ExternalOutput")

    with tile.TileContext(nc) as tc:
        with (
            tc.tile_pool(name="c1", bufs=1) as cp,
            tc.tile_pool(name="ps", bufs=2, space="PSUM") as pp,
        ):
            # hoist the activation-table load: dummy exp gated only on a
            # cheap DVE memset
            zz = cp.tile([1, 1], fp32, tag="zz")
            nc.vector.memset(zz[:], 0.0)
            zz2 = cp.tile([1, 1], fp32, tag="zz2")
            nc.scalar.activation(zz2[:], zz[:], AF.Exp)

            # input DMAs: s16 first on SP (tanh gates the ACT queue and the
            # DVE power chain), qpair via Pool SWDGE (parallel issue so the
            # PE broadcast no longer gates the exp), cst second on SP.
            # ACT queue stays clear of DMA issues so tanh dispatches ASAP.
            qp_sb = cp.tile([2, K], fp16, tag="qp")
            nc.gpsimd.dma_start(out=qp_sb[:], in_=qp_d[:])
            s16 = cp.tile([P, F], fp16, tag="s16")
            nc.sync.dma_start(out=s16[:], in_=s_d[:])
            cst = cp.tile([P, 8], fp32, tag="cst")
            nc.sync.dma_start(out=cst[:], in_=cst_d[:])

            ones = cp.tile([2, P], fp16, tag="ones")
            nc.gpsimd.memset(ones[:], 1.0)

            # q broadcast to all partitions (exact via hi/lo fp16 pair)
            q_ps = pp.tile([P, K], fp32, tag="qps")
            nc.tensor.matmul(q_ps[:], ones[:], qp_sb[:], start=True, stop=True)

            # ACT: warp first (gates the DVE power chain), then the node exp
            T = cp.tile([P, F], fp16, tag="T")
            nc.scalar.activation(T[:], s16[:], AF.Tanh, scale=float(WARP_A))
            E = cp.tile([P, K], fp32, tag="E")
            S0 = cp.tile([P, 1], fp32, tag="S0")
            nc.scalar.activation(E[:], q_ps[:], AF.Exp, scale=cst[:, 0:1],
                                 accum_out=S0[:])

            # DVE: u = t^2 in fp16 (overlaps the fit above)
            P2 = cp.tile([P, F], fp16, tag="P2")
            nc.vector.tensor_tensor(P2[:], T[:], T[:], OP.mult)

            # gate_n = S0p/(S0p+S0m), all on DVE back-to-back (no cross-engine
            # sem hops; Ln/Sigmoid on ACT would each force a 1.3us table swap).
            # Copy S0m to base partition 0 first: the ISA requires equal base
            # partitions when both ALU inputs live in SBUF.
            Sm = cp.tile([NN, 1], fp32, tag="Sm")
            nc.vector.tensor_copy(Sm[:], S0[NN:P, :])
            Ssum = cp.tile([NN, 1], fp32, tag="Ssum")
            nc.vector.tensor_tensor(Ssum[:], S0[0:NN, :], Sm[:], OP.add)
            Srec = cp.tile([NN, 1], fp32, tag="Srec")
            nc.vector.reciprocal(Srec[:], Ssum[:])
            gate = cp.tile([NN, 1], fp32, tag="gate")
            nc.vector.tensor_tensor(gate[:], S0[0:NN, :], Srec[:], OP.mult)
            # PE: node gates -> monomial coefficients (replicated on all
            # 128 partitions); the stationary side is the gate column
            # broadcast along the free dim via a stride-0 AP (no broadcast
            # copy needed)
            c_ps = pp.tile([P, D + 1], fp32, tag="cps")
            nc.tensor.matmul(c_ps[:], gate[:].broadcast_to((NN, P)),
                             cst[0:NN, 1:D + 2], start=True, stop=True)
            c_sb = cp.tile([P, D + 1], fp32, tag="csb")
            nc.vector.tensor_copy(c_sb[:], c_ps[:])
            # Horner in u = t^2 with odd/even interleave (D=5):
            #   p = L0 + u*(L1 + u*L2),  L_i = c_{2i} + c_{2i+1} t
            # ts ops in 4x fp16 mode; op order interleaves independent work
            # between dependent chain links (and splits the dependent tail
            # into column halves) so no op pays the ~95ns write-commit stall.
            L2 = cp.tile([P, F], fp16, tag="L2")
            nc.vector.tensor_scalar(out=L2[:], in0=T[:],
                                    scalar1=c_sb[:, 5:6], scalar2=c_sb[:, 4:5],
                                    op0=OP.mult, op1=OP.add)
            L1 = cp.tile([P, F], fp16, tag="L1")
            nc.vector.tensor_scalar(out=L1[:], in0=T[:],
                                    scalar1=c_sb[:, 3:4], scalar2=c_sb[:, 2:3],
                                    op0=OP.mult, op1=OP.add)
            X2 = cp.tile([P, F], fp16, tag="X2")
            nc.vector.tensor_tensor(X2[:], P2[:], L2[:], OP.mult)
            L0 = cp.tile([P, F], fp16, tag="L0")
            nc.vector.tensor_scalar(out=L0[:], in0=T[:],
                                    scalar1=c_sb[:, 1:2], scalar2=c_sb[:, 0:1],
                                    op0=OP.mult, op1=OP.add)
            X3 = cp.tile([P, F], fp16, tag="X3")
            X4 = cp.tile([P, F], fp16, tag="X4")
            X5 = cp.tile([P, F], fp16, tag="X5")
            outt = cp.tile([P, F], fp16, tag="outt")
            H = F // 2
            sla = slice(0, H)
            slb = slice(H, F)
            nc.vector.tensor_tensor(X3[:, sla], L1[:, sla], X2[:, sla], OP.add)
            nc.vector.tensor_tensor(X3[:, slb], L1[:, slb], X2[:, slb], OP.add)
            nc.vector.tensor_tensor(X4[:, sla], P2[:, sla], X3[:, sla], OP.mult)
            nc.vector.tensor_tensor(X4[:, slb], P2[:, slb], X3[:, slb], OP.mult)
            nc.vector.tensor_tensor(X5[:, sla], L0[:, sla], X4[:, sla], OP.add)
            nc.vector.tensor_tensor(X5[:, slb], L0[:, slb], X4[:, slb], OP.add)
            nc.vector.tensor_tensor(outt[:, sla], X5[:, sla], s16[:, sla],
                                    OP.mult)
            nc.vector.tensor_tensor(outt[:, slb], X5[:, slb], s16[:, slb],
                                    OP.mult)
            nc.sync.dma_start(out=y_d[:], in_=outt[:])

    nc.compile()
    return nc


def _get_nc(variant=None):
    global _NC
    if _NC is None:
        _NC = _build_nc()
    return _NC


def _in_maps(key, query):
    cst = _host_constants()
    s2 = key.reshape(B, J)
    h = J // 2
    maps = []
    for c in range(NCORES):
        b, half = divmod(c, 2)
        q = query[b].astype(np.float32)
        qhi = q.astype(np.float16)
        qlo = (q - qhi.astype(np.float32)).astype(np.float16)
        s16 = s2[b, half * h:(half + 1) * h].reshape(P, F).astype(np.float16)
        maps.append({
            "s16": np.ascontiguousarray(s16),
            "qpair": np.ascontiguousarray(np.stack([qhi, qlo], 0)),
            "cst": cst,
        })
    return maps


def kernel(key, query, _variant=None, _trace=False):
    key = np.ascontiguousarray(key, dtype=np.float32)
    query = np.ascontiguousarray(query, dtype=np.float32)
    nc = _get_nc()
    from concourse.bass_utils import run_bass_kernel_spmd

    res = run_bass_kernel_spmd(
        nc, _in_maps(key, query), list(range(NCORES)), trace=_trace
    )
    h = J // 2
    out = np.empty((B, J), np.float32)
    for c in range(NCORES):
        b, half = divmod(c, 2)
        out[b, half * h:(half + 1) * h] = \
            res.results[c]["y"].astype(np.float32).reshape(h)
    if _trace:
        kernel.last_results = res
    return out.reshape(key.shape)
